# revision 17
# baseline (speedup 1.0000x reference)
"""Trainium2 Bass kernel for nn_LogMM: out = log(max(x @ matrix, tiny)).

Reference math: y = einsum('bsk,km->bsm', x, matrix); big = (y>0); small = 1-big;
out = log(max(y,eps))*big + log(max(y,eps))*small == log(max(y, eps)).
(y_big == y_small numerically, and big+small == 1 elementwise.)

Sharding: data-parallel over batch B=8, one batch slice per NeuronCore;
matrix replicated. Zero communication.

Per-core kernel: x_b [2048, 1024] @ matrix [1024, 1024] -> log -> out_b.
The contraction dim k must live on SBUF partitions for both matmul operands;
matrix is already [k, m], x tiles are transposed on-chip via PE transpose.
"""

import os
from contextlib import ExitStack

import numpy as np

import concourse.bass as bass
import concourse.bacc as bacc
import concourse.mybir as mybir
import concourse.tile as tile
from concourse.bass_utils import run_bass_kernel_spmd
from concourse.masks import make_identity

B, S, K, M = 8, 2048, 1024, 1024
P = 128
N_CORES = 8

# matmul input dtype: "fp8dr" (fp8e4m3 DoubleRow, 0.5 cyc/row), "f32" (exact,
# 4 cyc/row), "f32r" (fp32 bits, 1 cyc/row at N>=256), "bf16" (cast, 1 cyc/row)
MM_DT = os.environ.get("LOGMM_DT", "fp8dr")
N_TILE = 512
# timing aid: repeat the whole per-core computation R times inside the NEFF
REPEAT = int(os.environ.get("LOGMM_REPEAT", "1"))


def _emit(ctx: ExitStack, tc: "tile.TileContext", out_ap, x_ap, mat_ap, mm_dt: str):
    nc = tc.nc
    S_TILES = S // P  # 16
    KO = K // P  # 8
    MO = M // N_TILE

    # dtype of the SBUF tiles fed to the accumulation matmuls
    if mm_dt == "bf16":
        mm_sb_dt = mybir.dt.bfloat16
    elif mm_dt == "f32r":
        mm_sb_dt = mybir.dt.float32r
    else:
        mm_sb_dt = mybir.dt.float32

    # x tiles are loaded [s, k] and transposed on PE; the transpose runs in
    # the load dtype (fp32 for f32/f32r, bf16 for bf16 via casting DMA).
    # LOGMM_TDT=f32r additionally runs the transposes themselves in f32r
    # (1.5 vs 2.0 cyc/row) by loading x as f32r via casting DMA.
    if mm_dt == "bf16":
        ld_dt = mybir.dt.bfloat16
    elif mm_dt == "f32r" and os.environ.get("LOGMM_TDT", "f32r") == "f32r":
        ld_dt = mybir.dt.float32r
    else:
        ld_dt = mybir.dt.float32

    const_pool = ctx.enter_context(tc.tile_pool(name="const", bufs=1))
    xin_pool = ctx.enter_context(tc.tile_pool(name="xin", bufs=int(os.environ.get("LOGMM_XIN","6"))))
    xt_pool = ctx.enter_context(tc.tile_pool(name="xt", bufs=int(os.environ.get("LOGMM_XT","5"))))
    ob_pool = ctx.enter_context(tc.tile_pool(name="ob", bufs=4))
    pst_pool = ctx.enter_context(tc.tile_pool(name="pst", bufs=int(os.environ.get("LOGMM_PST","4")), space="PSUM"))
    psm_pool = ctx.enter_context(tc.tile_pool(name="psm", bufs=int(os.environ.get("LOGMM_PSM","4")), space="PSUM"))

    if ld_dt == mybir.dt.float32r:
        # affine_select can't produce f32r; build fp32 identity and DVE-round
        ident_f32 = const_pool.tile([P, P], mybir.dt.float32)
        make_identity(nc, ident_f32)
        ident = const_pool.tile([P, P], ld_dt)
        nc.vector.tensor_copy(ident[:], ident_f32[:])
    else:
        ident = const_pool.tile([P, P], ld_dt)
        make_identity(nc, ident)

    mat_sb = const_pool.tile([P, KO, M], mm_sb_dt)
    mat_src = mat_ap.rearrange("(ko p) m -> p ko m", p=P)
    x_tiles: dict = {}

    def load_x(st, chunks=1):
        x_nat = xin_pool.tile([P, K], ld_dt)  # s on partitions, k free
        dma = (nc.scalar if os.environ.get("LOGMM_XQ","sp")=="act" else nc.sync) if ld_dt != mybir.dt.bfloat16 else nc.gpsimd
        if ld_dt == mybir.dt.float32r:
            x_ap_ld = x_ap.bitcast(mybir.dt.float32r)
        else:
            x_ap_ld = x_ap
        cw = K // chunks
        for c in range(chunks):
            dma.dma_start(
                x_nat[:, c * cw : (c + 1) * cw],
                x_ap_ld[st * P : (st + 1) * P, c * cw : (c + 1) * cw],
            )
        x_tiles[st] = x_nat

    def load_matrix():
        # matrix -> SBUF [P(k_inner), KO(k_outer), M]; chunked per ko so the
        # first matmuls aren't gated on the full 4MB transfer.
        if mm_sb_dt == mybir.dt.float32r and os.environ.get("LOGMM_MBC", "1") == "1":
            # bitcast the DRAM source to f32r and DMA straight into mat_sb:
            # drops the fp32 staging buffer and 16 DVE rounding copies from
            # each matmul's wait chain (PE truncates f32r on ingest anyway)
            mat_src_r = mat_src.bitcast(mybir.dt.float32r)
            for ko in range(KO):
                for h in range(2):
                    h_sl = slice(h * (M // 2), (h + 1) * (M // 2))
                    nc.sync.dma_start(mat_sb[:, ko, h_sl], mat_src_r[:, ko, h_sl])
        elif mm_sb_dt == mybir.dt.float32r:
            mat_stage = const_pool.tile([P, KO, M], mybir.dt.float32)
            for ko in range(KO):
                for h in range(2):
                    h_sl = slice(h * (M // 2), (h + 1) * (M // 2))
                    nc.sync.dma_start(mat_stage[:, ko, h_sl], mat_src[:, ko, h_sl])
                    # rounds fp32 -> fp32r as required by the BIR verifier
                    nc.vector.tensor_copy(mat_sb[:, ko, h_sl], mat_stage[:, ko, h_sl])
        else:
            dma = nc.sync if mm_sb_dt == mybir.dt.float32 else nc.gpsimd
            for ko in range(KO):
                dma.dma_start(mat_sb[:, ko, :], mat_src[:, ko, :])

    xT_tiles: dict = {}
    TB = 512 // P  # transposes per PSUM bank

    def transpose_batch(st, kb):
        # transpose 4 128x128 blocks of x tile st into one PSUM bank, then one
        # [128,512] PSUM->SBUF copy (which also rounds to the matmul dtype).
        x_nat = x_tiles[st]
        if st not in xT_tiles:
            xT_tiles[st] = xt_pool.tile([P, KO, P], mm_sb_dt, name="xT", tag="xT")
        xT = xT_tiles[st]
        ps = pst_pool.tile([P, TB, P], ld_dt)
        for kt in range(TB):
            ko = kb * TB + kt
            nc.tensor.transpose(
                ps[:, kt, :], x_nat[:, ko * P : (ko + 1) * P], ident[:]
            )
        nc.vector.tensor_copy(xT[:, kb * TB : (kb + 1) * TB, :], ps[:])
        if kb == KO // TB - 1:
            x_tiles.pop(st)

    def emit_transposes(st):
        for kb in range(KO // TB):
            transpose_batch(st, kb)

    def emit_mms(st, mo_inner, t_st=None, last=False):
        s_sl = slice(st * P, (st + 1) * P)
        xT = xT_tiles.pop(st)
        # transpose batches for tile t_st, interleaved into this MM stream so
        # PE can fill waits (matrix pacing early on, psum/DVE waits later)
        fillers = (
            [(t_st, kb) for kb in range(KO // TB)] if t_st is not None else []
        )

        def filler(ko):
            if fillers and ko % 2 == 1:
                transpose_batch(*fillers.pop(0))

        def fin(mo, pm):
            m_sl = slice(mo * N_TILE, (mo + 1) * N_TILE)
            ob = ob_pool.tile([P, N_TILE], mybir.dt.float32)
            nc.scalar.activation(ob[:], pm[:], mybir.ActivationFunctionType.Ln)
            nc.sync.dma_start(out_ap[s_sl, m_sl], ob[:])

        if mo_inner:
            # each matmul gates on a single matrix ko-chunk (matters for the
            # first s-tiles while the matrix is still streaming in)
            pms = [
                psm_pool.tile([P, N_TILE], mybir.dt.float32, name=f"pm{mo}", tag="pm")
                for mo in range(MO)
            ]
            for ko in range(KO):
                for mo in range(MO):
                    nc.tensor.matmul(
                        pms[mo][:],
                        xT[:, ko, :],
                        mat_sb[:, ko, mo * N_TILE : (mo + 1) * N_TILE],
                        start=(ko == 0),
                        stop=(ko == KO - 1),
                    )
                filler(ko)
            for mo in range(MO):
                fin(mo, pms[mo])
        else:
            # mo-outer: each psum finishes asap so log+store drain earlier
            for mo in range(MO):
                pm = psm_pool.tile([P, N_TILE], mybir.dt.float32, tag="pm")
                for ko in range(KO):
                    nc.tensor.matmul(
                        pm[:],
                        xT[:, ko, :],
                        mat_sb[:, ko, mo * N_TILE : (mo + 1) * N_TILE],
                        start=(ko == 0),
                        stop=(ko == KO - 1),
                    )
                    filler(mo * KO + ko)
                fin(mo, pm)

    DEPTH = int(os.environ.get("LOGMM_DEPTH", "3"))

    def body(_i=None):  # noqa: C901
        next_load = 0

        def ensure_x(up_to):
            nonlocal next_load
            while next_load <= min(up_to, S_TILES - 1):
                # first tiles in small chunks so the first transposes start asap
                load_x(next_load)
                next_load += 1

        # first x tiles before the matrix so PE transposes start immediately
        ensure_x(1)
        load_matrix()
        # HAM pre-warm: the PE would otherwise idle ~4us waiting for x tile 0
        # and then run its first ~3.4us of real work at the cold 1.2 GHz clock.
        # Dummy transposes of the (already resident) identity keep the PE busy
        # through the DMA wait so real work starts at 2.4 GHz.
        n_warm = int(os.environ.get("LOGMM_WARM", "0"))
        if n_warm:
            ps_w = pst_pool.tile([P, TB, P], ld_dt, name="ps_warm", tag="ps")
            for w in range(n_warm):
                nc.tensor.transpose(ps_w[:, w % TB, :], ident[:], ident[:])
            # consume the last dummy so DCE keeps the chain: store one row
            # into out[0:P, 0:P], which s-tile 0's real store later overwrites
            warm_sb = ob_pool.tile([P, P], ld_dt, name="warm_sb")
            nc.vector.tensor_copy(warm_sb[:], ps_w[:, 0, :])
            nc.sync.dma_start(
                out_ap[0:P, 0:P], warm_sb[:].bitcast(mybir.dt.float32)
            )
        for st in range(DEPTH):
            ensure_x(st + 2)
            emit_transposes(st)
        for st in range(S_TILES):
            t_st = st + DEPTH if st + DEPTH < S_TILES else None
            if t_st is not None:
                ensure_x(t_st + 2)
            emit_mms(st, mo_inner=st < int(os.environ.get("LOGMM_MOI","2")), t_st=t_st, last=st >= S_TILES - 2)

    if REPEAT > 1:
        with tc.For_i(0, REPEAT, 1) as _i:
            body(_i)
    else:
        body()


def _emit_fp8(ctx: ExitStack, tc: "tile.TileContext", out_ap, x_ap, mat_ap):
    """fp8e4m3 DoubleRow pipeline.

    x, matrix are cast fp32->{bf16,fp8} inline by SWDGE (gpsimd) DMAs, which
    also moves all loads off the HWDGE ring (stores keep it). x tiles are
    PE-transposed (1 cyc/row at 16-bit/8-bit), DVE-copied into fp8 xT tiles,
    then each s-tile runs KP=4 DoubleRow matmuls per 512-wide output half:
    contraction 256 per matmul via the [ki, 2, *] interleaved APs on both
    operands. PSUM accumulates fp32; ACT applies Ln; sync HWDGE stores.
    """
    nc = tc.nc
    S_TILES = S // P  # 16
    KO = K // P  # 8
    KP = KO // 2  # DoubleRow k-pair groups
    MO = M // N_TILE

    mm_dt = mybir.dt.float8e4
    # x load dtype:
    #  - f32r (default): plain HWDGE load on the sync ring, PE transpose at
    #    1.5 cyc/row, fp8 conversion folded into the DVE PSUM->SBUF copy.
    #    Keeps the Q7/SWDGE descriptor engine (which shares an SBUF port
    #    with DVE) out of the x path entirely.
    #  - bf16/fp8: SWDGE casting DMA on gpsimd, 1 cyc/row transpose.
    XDT = os.environ.get("LOGMM_XDT", "f32r")
    ld_dt = {
        "fp8": mybir.dt.float8e4,
        "bf16": mybir.dt.bfloat16,
        "f32r": mybir.dt.float32r,
    }[XDT]

    const_pool = ctx.enter_context(tc.tile_pool(name="const", bufs=1))
    # matrix double-buffered across REPEAT iterations: iteration i+1's reload
    # must not WAR-stall on iteration i's last matmuls
    mat_pool = ctx.enter_context(
        tc.tile_pool(name="matp", bufs=int(os.environ.get("LOGMM_MATB", "2")))
    )
    xin_pool = ctx.enter_context(
        tc.tile_pool(name="xin", bufs=int(os.environ.get("LOGMM_XIN", "8")))
    )
    xt_pool = ctx.enter_context(
        tc.tile_pool(name="xt", bufs=int(os.environ.get("LOGMM_XT", "5")))
    )
    ob_pool = ctx.enter_context(tc.tile_pool(name="ob", bufs=4))
    pst_pool = ctx.enter_context(
        tc.tile_pool(name="pst", bufs=int(os.environ.get("LOGMM_PST", "3")), space="PSUM")
    )
    psm_pool = ctx.enter_context(
        tc.tile_pool(name="psm", bufs=int(os.environ.get("LOGMM_PSM", "5")), space="PSUM")
    )

    if ld_dt == mybir.dt.float32r:
        # affine_select can't produce f32r; build fp32 identity and DVE-round
        ident_f32 = const_pool.tile([P, P], mybir.dt.float32)
        make_identity(nc, ident_f32)
        ident = const_pool.tile([P, P], ld_dt)
        nc.vector.tensor_copy(ident[:], ident_f32[:])
    else:
        ident = const_pool.tile([P, P], ld_dt)
        make_identity(nc, ident)

    mat_src = mat_ap.rearrange("(ko p) m -> p ko m", p=P)
    x_tiles: dict = {}
    mat_sb = None
    stq = os.environ.get("LOGMM_STQ", "scalar" if XDT == "f32r" else "sync")
    store_engines = {
        "alt": (nc.sync, nc.scalar),
        "scalar": (nc.scalar,),
        "sync": (nc.sync,),
    }[stq]

    XCH = int(os.environ.get("LOGMM_XCH", "1"))  # s-tiles per x load DMA
    x_ap_ld = (
        x_ap.bitcast(mybir.dt.float32r) if ld_dt == mybir.dt.float32r else x_ap
    )
    x_src_g = x_ap_ld.rearrange("(st g p) k -> st p g k", p=P, g=XCH)

    def load_x(st0):
        # one DMA covering XCH s-tiles: [128, XCH, K]; slice g recovers
        # s-tile st0+g. f32r goes over the sync HWDGE ring (no cast); the
        # 16-bit/8-bit dtypes need the SWDGE (gpsimd) casting path.
        x_nat = xin_pool.tile([P, XCH, K], ld_dt)
        dma = nc.sync if ld_dt == mybir.dt.float32r else nc.gpsimd
        dma.dma_start(x_nat[:], x_src_g[st0 // XCH])
        for g in range(XCH):
            x_tiles[st0 + g] = x_nat[:, g, :]

    MATLD = os.environ.get("LOGMM_MATLD", "swdge")
    mat_stage_pool = (
        ctx.enter_context(tc.tile_pool(name="mats", bufs=2))
        if MATLD == "hwdge"
        else None
    )

    def load_matrix():
        # fp32 -> fp8; chunked so the first matmuls only gate on the ko-pairs
        # they consume. swdge: casting DMA on gpsimd. hwdge: plain fp32 load
        # on the sync ring + DVE conversion (keeps Q7/SWDGE fully idle).
        nonlocal mat_sb
        mat_sb = mat_pool.tile([P, KO, M], mm_dt, name="mat", tag="mat")
        mch = int(os.environ.get("LOGMM_MATCHUNK", "8"))
        per = KO // mch
        for c in range(mch):
            c_sl = slice(c * per, (c + 1) * per)
            if MATLD == "hwdge":
                stage = mat_stage_pool.tile([P, per, M], mybir.dt.float32)
                nc.sync.dma_start(stage[:], mat_src[:, c_sl, :])
                nc.vector.tensor_copy(mat_sb[:, c_sl, :], stage[:])
            else:
                nc.gpsimd.dma_start(mat_sb[:, c_sl, :], mat_src[:, c_sl, :])

    xT_tiles: dict = {}
    TB = 512 // P  # transposes per PSUM bank

    def transpose_batch(st, kb):
        x_nat = x_tiles[st]
        if st not in xT_tiles:
            xT_tiles[st] = xt_pool.tile([P, KO, P], mm_dt, name="xT", tag="xT")
        xT = xT_tiles[st]
        ps = pst_pool.tile([P, TB, P], ld_dt)
        for kt in range(TB):
            ko = kb * TB + kt
            nc.tensor.transpose(
                ps[:, kt, :], x_nat[:, ko * P : (ko + 1) * P], ident[:]
            )
        nc.vector.tensor_copy(xT[:, kb * TB : (kb + 1) * TB, :], ps[:])
        if kb == KO // TB - 1:
            x_tiles.pop(st)

    def emit_transposes(st):
        for kb in range(KO // TB):
            transpose_batch(st, kb)

    def emit_mms(st, t_st=None):
        s_sl = slice(st * P, (st + 1) * P)
        xT = xT_tiles.pop(st)
        fillers = (
            [(t_st, kb) for kb in range(KO // TB)] if t_st is not None else []
        )

        def filler():
            if fillers:
                transpose_batch(*fillers.pop(0))

        pms = [
            psm_pool.tile([P, N_TILE], mybir.dt.float32, name=f"pm{mo}", tag="pm")
            for mo in range(MO)
        ]
        for j in range(KP):
            lhsT = xT[:, 2 * j : 2 * j + 2, :]
            for mo in range(MO):
                nc.tensor.matmul(
                    pms[mo][:],
                    lhsT,
                    mat_sb[:, 2 * j : 2 * j + 2, mo * N_TILE : (mo + 1) * N_TILE],
                    start=(j == 0),
                    stop=(j == KP - 1),
                    perf_mode=mybir.MatmulPerfMode.DoubleRow,
                )
            if j % 2 == 1:
                filler()
        if os.environ.get("LOGMM_STMERGE", "0") == "1":
            # both Ln halves into one [P, M] tile, single 512KB store per
            # s-tile — halves the HWDGE store issue count
            ob = ob_pool.tile([P, M], mybir.dt.float32)
            for mo in range(MO):
                m_sl = slice(mo * N_TILE, (mo + 1) * N_TILE)
                nc.scalar.activation(
                    ob[:, m_sl], pms[mo][:], mybir.ActivationFunctionType.Ln
                )
            store_engines[st % len(store_engines)].dma_start(out_ap[s_sl, :], ob[:])
        else:
            for mo in range(MO):
                m_sl = slice(mo * N_TILE, (mo + 1) * N_TILE)
                ob = ob_pool.tile([P, N_TILE], mybir.dt.float32)
                nc.scalar.activation(
                    ob[:], pms[mo][:], mybir.ActivationFunctionType.Ln
                )
                store_engines[(st * MO + mo) % len(store_engines)].dma_start(
                    out_ap[s_sl, m_sl], ob[:]
                )

    DEPTH = int(os.environ.get("LOGMM_DEPTH", "3"))
    # LOGMM_DIAG=noload: hoist all loads out of the repeat loop (needs
    # LOGMM_XIN=16) — isolates the compute+store pipeline for HW timing
    DIAG = os.environ.get("LOGMM_DIAG", "")

    def body(_i=None):
        next_load = 0

        def ensure_x(up_to):
            nonlocal next_load
            if DIAG == "noload":
                return
            while next_load <= min(up_to, S_TILES - 1):
                load_x(next_load)
                next_load += XCH

        ensure_x(1)
        if DIAG != "noload":
            load_matrix()
        for st in range(DEPTH):
            ensure_x(st + 2)
            emit_transposes(st)
        for st in range(S_TILES):
            t_st = st + DEPTH if st + DEPTH < S_TILES else None
            if t_st is not None:
                ensure_x(t_st + 2)
            emit_mms(st, t_st=t_st)

    if DIAG == "noload":
        for st in range(0, S_TILES, XCH):
            load_x(st)
        load_matrix()

        # x_tiles entries are popped by the transposes each iteration; keep a
        # pristine copy to re-seed per iteration
        x_tiles_full = dict(x_tiles)

        def body_noload(_i=None):
            x_tiles.clear()
            x_tiles.update(x_tiles_full)
            body(_i)

        if REPEAT > 1:
            with tc.For_i(0, REPEAT, 1) as _i:
                body_noload(_i)
        else:
            body_noload()
    elif REPEAT > 1:
        with tc.For_i(0, REPEAT, 1) as _i:
            body(_i)
    else:
        body()


def _build_nc(mm_dt: str):
    nc = bacc.Bacc("TRN2", target_bir_lowering=False, debug=False)
    x = nc.dram_tensor("x", [S, K], mybir.dt.float32, kind="ExternalInput").ap()
    mat = nc.dram_tensor("matrix", [K, M], mybir.dt.float32, kind="ExternalInput").ap()
    out = nc.dram_tensor("out", [S, M], mybir.dt.float32, kind="ExternalOutput").ap()
    with tile.TileContext(nc) as tc:
        with ExitStack() as ctx:
            if mm_dt == "fp8dr":
                _emit_fp8(ctx, tc, out, x, mat)
            else:
                _emit(ctx, tc, out, x, mat, mm_dt)
    nc.compile()
    return nc


_nc_cache: dict = {}


def _get_nc(mm_dt: str):
    if mm_dt not in _nc_cache:
        _nc_cache[mm_dt] = _build_nc(mm_dt)
    return _nc_cache[mm_dt]


def kernel(x: np.ndarray, matrix: np.ndarray, _trace: bool = False):
    assert x.shape == (B, S, K) and matrix.shape == (K, M)
    nc = _get_nc(MM_DT)
    x = np.ascontiguousarray(x, dtype=np.float32)
    matrix = np.ascontiguousarray(matrix, dtype=np.float32)
    in_maps = [{"x": x[b], "matrix": matrix} for b in range(N_CORES)]
    res = run_bass_kernel_spmd(nc, in_maps, core_ids=list(range(N_CORES)), trace=_trace)
    out = np.stack([r["out"] for r in res.results], axis=0)
    if _trace:
        kernel.last_results = res  # stash for profiling inspection
    return out



# revision 18
# speedup vs baseline: 1.0892x; 1.0892x over previous
"""Trainium2 Bass kernel for nn_LogMM: out = log(max(x @ matrix, tiny)).

Reference math: y = einsum('bsk,km->bsm', x, matrix); big = (y>0); small = 1-big;
out = log(max(y,eps))*big + log(max(y,eps))*small == log(max(y, eps)).
(y_big == y_small numerically, and big+small == 1 elementwise.)

Sharding: data-parallel over batch B=8, one batch slice per NeuronCore;
matrix replicated. Zero communication.

Per-core kernel: x_b [2048, 1024] @ matrix [1024, 1024] -> log -> out_b.
The contraction dim k must live on SBUF partitions for both matmul operands;
matrix is already [k, m], x tiles are transposed on-chip via PE transpose.
"""

import os
from contextlib import ExitStack

import numpy as np

import concourse.bass as bass
import concourse.bacc as bacc
import concourse.mybir as mybir
import concourse.tile as tile
from concourse.bass_utils import run_bass_kernel_spmd
from concourse.masks import make_identity

B, S, K, M = 8, 2048, 1024, 1024
P = 128
N_CORES = 8

# matmul input dtype: "fp8dr" (fp8e4m3 DoubleRow, 0.5 cyc/row), "f32" (exact,
# 4 cyc/row), "f32r" (fp32 bits, 1 cyc/row at N>=256), "bf16" (cast, 1 cyc/row)
MM_DT = os.environ.get("LOGMM_DT", "fp8dr")
N_TILE = 512
# timing aid: repeat the whole per-core computation R times inside the NEFF
REPEAT = int(os.environ.get("LOGMM_REPEAT", "1"))


def _emit(ctx: ExitStack, tc: "tile.TileContext", out_ap, x_ap, mat_ap, mm_dt: str):
    nc = tc.nc
    S_TILES = S // P  # 16
    KO = K // P  # 8
    MO = M // N_TILE

    # dtype of the SBUF tiles fed to the accumulation matmuls
    if mm_dt == "bf16":
        mm_sb_dt = mybir.dt.bfloat16
    elif mm_dt == "f32r":
        mm_sb_dt = mybir.dt.float32r
    else:
        mm_sb_dt = mybir.dt.float32

    # x tiles are loaded [s, k] and transposed on PE; the transpose runs in
    # the load dtype (fp32 for f32/f32r, bf16 for bf16 via casting DMA).
    # LOGMM_TDT=f32r additionally runs the transposes themselves in f32r
    # (1.5 vs 2.0 cyc/row) by loading x as f32r via casting DMA.
    if mm_dt == "bf16":
        ld_dt = mybir.dt.bfloat16
    elif mm_dt == "f32r" and os.environ.get("LOGMM_TDT", "f32r") == "f32r":
        ld_dt = mybir.dt.float32r
    else:
        ld_dt = mybir.dt.float32

    const_pool = ctx.enter_context(tc.tile_pool(name="const", bufs=1))
    xin_pool = ctx.enter_context(tc.tile_pool(name="xin", bufs=int(os.environ.get("LOGMM_XIN","6"))))
    xt_pool = ctx.enter_context(tc.tile_pool(name="xt", bufs=int(os.environ.get("LOGMM_XT","5"))))
    ob_pool = ctx.enter_context(tc.tile_pool(name="ob", bufs=4))
    pst_pool = ctx.enter_context(tc.tile_pool(name="pst", bufs=int(os.environ.get("LOGMM_PST","4")), space="PSUM"))
    psm_pool = ctx.enter_context(tc.tile_pool(name="psm", bufs=int(os.environ.get("LOGMM_PSM","4")), space="PSUM"))

    if ld_dt == mybir.dt.float32r:
        # affine_select can't produce f32r; build fp32 identity and DVE-round
        ident_f32 = const_pool.tile([P, P], mybir.dt.float32)
        make_identity(nc, ident_f32)
        ident = const_pool.tile([P, P], ld_dt)
        nc.vector.tensor_copy(ident[:], ident_f32[:])
    else:
        ident = const_pool.tile([P, P], ld_dt)
        make_identity(nc, ident)

    mat_sb = const_pool.tile([P, KO, M], mm_sb_dt)
    mat_src = mat_ap.rearrange("(ko p) m -> p ko m", p=P)
    x_tiles: dict = {}

    def load_x(st, chunks=1):
        x_nat = xin_pool.tile([P, K], ld_dt)  # s on partitions, k free
        dma = (nc.scalar if os.environ.get("LOGMM_XQ","sp")=="act" else nc.sync) if ld_dt != mybir.dt.bfloat16 else nc.gpsimd
        if ld_dt == mybir.dt.float32r:
            x_ap_ld = x_ap.bitcast(mybir.dt.float32r)
        else:
            x_ap_ld = x_ap
        cw = K // chunks
        for c in range(chunks):
            dma.dma_start(
                x_nat[:, c * cw : (c + 1) * cw],
                x_ap_ld[st * P : (st + 1) * P, c * cw : (c + 1) * cw],
            )
        x_tiles[st] = x_nat

    def load_matrix():
        # matrix -> SBUF [P(k_inner), KO(k_outer), M]; chunked per ko so the
        # first matmuls aren't gated on the full 4MB transfer.
        if mm_sb_dt == mybir.dt.float32r and os.environ.get("LOGMM_MBC", "1") == "1":
            # bitcast the DRAM source to f32r and DMA straight into mat_sb:
            # drops the fp32 staging buffer and 16 DVE rounding copies from
            # each matmul's wait chain (PE truncates f32r on ingest anyway)
            mat_src_r = mat_src.bitcast(mybir.dt.float32r)
            for ko in range(KO):
                for h in range(2):
                    h_sl = slice(h * (M // 2), (h + 1) * (M // 2))
                    nc.sync.dma_start(mat_sb[:, ko, h_sl], mat_src_r[:, ko, h_sl])
        elif mm_sb_dt == mybir.dt.float32r:
            mat_stage = const_pool.tile([P, KO, M], mybir.dt.float32)
            for ko in range(KO):
                for h in range(2):
                    h_sl = slice(h * (M // 2), (h + 1) * (M // 2))
                    nc.sync.dma_start(mat_stage[:, ko, h_sl], mat_src[:, ko, h_sl])
                    # rounds fp32 -> fp32r as required by the BIR verifier
                    nc.vector.tensor_copy(mat_sb[:, ko, h_sl], mat_stage[:, ko, h_sl])
        else:
            dma = nc.sync if mm_sb_dt == mybir.dt.float32 else nc.gpsimd
            for ko in range(KO):
                dma.dma_start(mat_sb[:, ko, :], mat_src[:, ko, :])

    xT_tiles: dict = {}
    TB = 512 // P  # transposes per PSUM bank

    def transpose_batch(st, kb):
        # transpose 4 128x128 blocks of x tile st into one PSUM bank, then one
        # [128,512] PSUM->SBUF copy (which also rounds to the matmul dtype).
        x_nat = x_tiles[st]
        if st not in xT_tiles:
            xT_tiles[st] = xt_pool.tile([P, KO, P], mm_sb_dt, name="xT", tag="xT")
        xT = xT_tiles[st]
        ps = pst_pool.tile([P, TB, P], ld_dt)
        for kt in range(TB):
            ko = kb * TB + kt
            nc.tensor.transpose(
                ps[:, kt, :], x_nat[:, ko * P : (ko + 1) * P], ident[:]
            )
        nc.vector.tensor_copy(xT[:, kb * TB : (kb + 1) * TB, :], ps[:])
        if kb == KO // TB - 1:
            x_tiles.pop(st)

    def emit_transposes(st):
        for kb in range(KO // TB):
            transpose_batch(st, kb)

    def emit_mms(st, mo_inner, t_st=None, last=False):
        s_sl = slice(st * P, (st + 1) * P)
        xT = xT_tiles.pop(st)
        # transpose batches for tile t_st, interleaved into this MM stream so
        # PE can fill waits (matrix pacing early on, psum/DVE waits later)
        fillers = (
            [(t_st, kb) for kb in range(KO // TB)] if t_st is not None else []
        )

        def filler(ko):
            if fillers and ko % 2 == 1:
                transpose_batch(*fillers.pop(0))

        def fin(mo, pm):
            m_sl = slice(mo * N_TILE, (mo + 1) * N_TILE)
            ob = ob_pool.tile([P, N_TILE], mybir.dt.float32)
            nc.scalar.activation(ob[:], pm[:], mybir.ActivationFunctionType.Ln)
            nc.sync.dma_start(out_ap[s_sl, m_sl], ob[:])

        if mo_inner:
            # each matmul gates on a single matrix ko-chunk (matters for the
            # first s-tiles while the matrix is still streaming in)
            pms = [
                psm_pool.tile([P, N_TILE], mybir.dt.float32, name=f"pm{mo}", tag="pm")
                for mo in range(MO)
            ]
            for ko in range(KO):
                for mo in range(MO):
                    nc.tensor.matmul(
                        pms[mo][:],
                        xT[:, ko, :],
                        mat_sb[:, ko, mo * N_TILE : (mo + 1) * N_TILE],
                        start=(ko == 0),
                        stop=(ko == KO - 1),
                    )
                filler(ko)
            for mo in range(MO):
                fin(mo, pms[mo])
        else:
            # mo-outer: each psum finishes asap so log+store drain earlier
            for mo in range(MO):
                pm = psm_pool.tile([P, N_TILE], mybir.dt.float32, tag="pm")
                for ko in range(KO):
                    nc.tensor.matmul(
                        pm[:],
                        xT[:, ko, :],
                        mat_sb[:, ko, mo * N_TILE : (mo + 1) * N_TILE],
                        start=(ko == 0),
                        stop=(ko == KO - 1),
                    )
                    filler(mo * KO + ko)
                fin(mo, pm)

    DEPTH = int(os.environ.get("LOGMM_DEPTH", "3"))

    def body(_i=None):  # noqa: C901
        next_load = 0

        def ensure_x(up_to):
            nonlocal next_load
            while next_load <= min(up_to, S_TILES - 1):
                # first tiles in small chunks so the first transposes start asap
                load_x(next_load)
                next_load += 1

        # first x tiles before the matrix so PE transposes start immediately
        ensure_x(1)
        load_matrix()
        # HAM pre-warm: the PE would otherwise idle ~4us waiting for x tile 0
        # and then run its first ~3.4us of real work at the cold 1.2 GHz clock.
        # Dummy transposes of the (already resident) identity keep the PE busy
        # through the DMA wait so real work starts at 2.4 GHz.
        n_warm = int(os.environ.get("LOGMM_WARM", "0"))
        if n_warm:
            ps_w = pst_pool.tile([P, TB, P], ld_dt, name="ps_warm", tag="ps")
            for w in range(n_warm):
                nc.tensor.transpose(ps_w[:, w % TB, :], ident[:], ident[:])
            # consume the last dummy so DCE keeps the chain: store one row
            # into out[0:P, 0:P], which s-tile 0's real store later overwrites
            warm_sb = ob_pool.tile([P, P], ld_dt, name="warm_sb")
            nc.vector.tensor_copy(warm_sb[:], ps_w[:, 0, :])
            nc.sync.dma_start(
                out_ap[0:P, 0:P], warm_sb[:].bitcast(mybir.dt.float32)
            )
        for st in range(DEPTH):
            ensure_x(st + 2)
            emit_transposes(st)
        for st in range(S_TILES):
            t_st = st + DEPTH if st + DEPTH < S_TILES else None
            if t_st is not None:
                ensure_x(t_st + 2)
            emit_mms(st, mo_inner=st < int(os.environ.get("LOGMM_MOI","2")), t_st=t_st, last=st >= S_TILES - 2)

    if REPEAT > 1:
        with tc.For_i(0, REPEAT, 1) as _i:
            body(_i)
    else:
        body()


def _emit_fp8(ctx: ExitStack, tc: "tile.TileContext", out_ap, x_ap, mat_ap):
    """fp8e4m3 DoubleRow pipeline.

    x, matrix are cast fp32->{bf16,fp8} inline by SWDGE (gpsimd) DMAs, which
    also moves all loads off the HWDGE ring (stores keep it). x tiles are
    PE-transposed (1 cyc/row at 16-bit/8-bit), DVE-copied into fp8 xT tiles,
    then each s-tile runs KP=4 DoubleRow matmuls per 512-wide output half:
    contraction 256 per matmul via the [ki, 2, *] interleaved APs on both
    operands. PSUM accumulates fp32; ACT applies Ln; sync HWDGE stores.
    """
    nc = tc.nc
    S_TILES = S // P  # 16
    KO = K // P  # 8
    KP = KO // 2  # DoubleRow k-pair groups
    MO = M // N_TILE

    mm_dt = mybir.dt.float8e4
    # x load dtype:
    #  - f32r (default): plain HWDGE load on the sync ring, PE transpose at
    #    1.5 cyc/row, fp8 conversion folded into the DVE PSUM->SBUF copy.
    #    Keeps the Q7/SWDGE descriptor engine (which shares an SBUF port
    #    with DVE) out of the x path entirely.
    #  - bf16/fp8: SWDGE casting DMA on gpsimd, 1 cyc/row transpose.
    XDT = os.environ.get("LOGMM_XDT", "f32r")
    ld_dt = {
        "fp8": mybir.dt.float8e4,
        "bf16": mybir.dt.bfloat16,
        "f32r": mybir.dt.float32r,
    }[XDT]

    const_pool = ctx.enter_context(tc.tile_pool(name="const", bufs=1))
    # matrix double-buffered across REPEAT iterations: iteration i+1's reload
    # must not WAR-stall on iteration i's last matmuls
    mat_pool = ctx.enter_context(
        tc.tile_pool(name="matp", bufs=int(os.environ.get("LOGMM_MATB", "2")))
    )
    xin_pool = ctx.enter_context(
        tc.tile_pool(name="xin", bufs=int(os.environ.get("LOGMM_XIN", "8")))
    )
    xt_pool = ctx.enter_context(
        tc.tile_pool(name="xt", bufs=int(os.environ.get("LOGMM_XT", "5")))
    )
    ob_pool = ctx.enter_context(
        tc.tile_pool(name="ob", bufs=int(os.environ.get("LOGMM_OB", "4")))
    )
    pst_pool = ctx.enter_context(
        tc.tile_pool(name="pst", bufs=int(os.environ.get("LOGMM_PST", "3")), space="PSUM")
    )
    psm_pool = ctx.enter_context(
        tc.tile_pool(name="psm", bufs=int(os.environ.get("LOGMM_PSM", "5")), space="PSUM")
    )

    if ld_dt == mybir.dt.float32r:
        # affine_select can't produce f32r; build fp32 identity and DVE-round
        ident_f32 = const_pool.tile([P, P], mybir.dt.float32)
        make_identity(nc, ident_f32)
        ident = const_pool.tile([P, P], ld_dt)
        nc.vector.tensor_copy(ident[:], ident_f32[:])
    else:
        ident = const_pool.tile([P, P], ld_dt)
        make_identity(nc, ident)

    mat_src = mat_ap.rearrange("(ko p) m -> p ko m", p=P)
    x_tiles: dict = {}
    mat_sb = None
    stq = os.environ.get("LOGMM_STQ", "scalar" if XDT == "f32r" else "sync")
    store_engines = {
        "alt": (nc.sync, nc.scalar),
        "scalar": (nc.scalar,),
        "sync": (nc.sync,),
    }[stq]

    XCH = int(os.environ.get("LOGMM_XCH", "1"))  # s-tiles per x load DMA
    x_ap_ld = (
        x_ap.bitcast(mybir.dt.float32r) if ld_dt == mybir.dt.float32r else x_ap
    )
    x_src_g = x_ap_ld.rearrange("(st g p) k -> st p g k", p=P, g=XCH)

    def load_x(st0):
        # one DMA covering XCH s-tiles: [128, XCH, K]; slice g recovers
        # s-tile st0+g. f32r goes over the sync HWDGE ring (no cast); the
        # 16-bit/8-bit dtypes need the SWDGE (gpsimd) casting path.
        x_nat = xin_pool.tile([P, XCH, K], ld_dt)
        dma = nc.sync if ld_dt == mybir.dt.float32r else nc.gpsimd
        dma.dma_start(x_nat[:], x_src_g[st0 // XCH])
        for g in range(XCH):
            x_tiles[st0 + g] = x_nat[:, g, :]

    MATLD = os.environ.get("LOGMM_MATLD", "swdge")
    mat_stage_pool = (
        ctx.enter_context(tc.tile_pool(name="mats", bufs=2))
        if MATLD == "hwdge"
        else None
    )

    def load_matrix():
        # fp32 -> fp8; chunked so the first matmuls only gate on the ko-pairs
        # they consume. swdge: casting DMA on gpsimd. hwdge: plain fp32 load
        # on the sync ring + DVE conversion (keeps Q7/SWDGE fully idle).
        nonlocal mat_sb
        mat_sb = mat_pool.tile([P, KO, M], mm_dt, name="mat", tag="mat")
        mch = int(os.environ.get("LOGMM_MATCHUNK", "8"))
        per = KO // mch
        for c in range(mch):
            c_sl = slice(c * per, (c + 1) * per)
            if MATLD == "hwdge":
                stage = mat_stage_pool.tile([P, per, M], mybir.dt.float32)
                nc.sync.dma_start(stage[:], mat_src[:, c_sl, :])
                nc.vector.tensor_copy(mat_sb[:, c_sl, :], stage[:])
            else:
                nc.gpsimd.dma_start(mat_sb[:, c_sl, :], mat_src[:, c_sl, :])

    xT_tiles: dict = {}
    TB = 512 // P  # transposes per PSUM bank

    def transpose_batch(st, kb):
        x_nat = x_tiles[st]
        if st not in xT_tiles:
            xT_tiles[st] = xt_pool.tile([P, KO, P], mm_dt, name="xT", tag="xT")
        xT = xT_tiles[st]
        ps = pst_pool.tile([P, TB, P], ld_dt)
        for kt in range(TB):
            ko = kb * TB + kt
            nc.tensor.transpose(
                ps[:, kt, :], x_nat[:, ko * P : (ko + 1) * P], ident[:]
            )
        nc.vector.tensor_copy(xT[:, kb * TB : (kb + 1) * TB, :], ps[:])
        if kb == KO // TB - 1:
            x_tiles.pop(st)

    def emit_transposes(st):
        for kb in range(KO // TB):
            transpose_batch(st, kb)

    def emit_mms(st, t_st=None):
        s_sl = slice(st * P, (st + 1) * P)
        xT = xT_tiles.pop(st)
        fillers = (
            [(t_st, kb) for kb in range(KO // TB)] if t_st is not None else []
        )

        def filler():
            if fillers:
                transpose_batch(*fillers.pop(0))

        pms = [
            psm_pool.tile([P, N_TILE], mybir.dt.float32, name=f"pm{mo}", tag="pm")
            for mo in range(MO)
        ]
        for j in range(KP):
            lhsT = xT[:, 2 * j : 2 * j + 2, :]
            for mo in range(MO):
                nc.tensor.matmul(
                    pms[mo][:],
                    lhsT,
                    mat_sb[:, 2 * j : 2 * j + 2, mo * N_TILE : (mo + 1) * N_TILE],
                    start=(j == 0),
                    stop=(j == KP - 1),
                    perf_mode=mybir.MatmulPerfMode.DoubleRow,
                )
            if j % 2 == 1:
                filler()
        if os.environ.get("LOGMM_STMERGE", "0") == "1":
            # both Ln halves into one [P, M] tile, single 512KB store per
            # s-tile — halves the HWDGE store issue count
            ob = ob_pool.tile([P, M], mybir.dt.float32)
            for mo in range(MO):
                m_sl = slice(mo * N_TILE, (mo + 1) * N_TILE)
                nc.scalar.activation(
                    ob[:, m_sl], pms[mo][:], mybir.ActivationFunctionType.Ln
                )
            store_engines[st % len(store_engines)].dma_start(out_ap[s_sl, :], ob[:])
        else:
            for mo in range(MO):
                m_sl = slice(mo * N_TILE, (mo + 1) * N_TILE)
                ob = ob_pool.tile([P, N_TILE], mybir.dt.float32)
                nc.scalar.activation(
                    ob[:], pms[mo][:], mybir.ActivationFunctionType.Ln
                )
                store_engines[(st * MO + mo) % len(store_engines)].dma_start(
                    out_ap[s_sl, m_sl], ob[:]
                )

    DEPTH = int(os.environ.get("LOGMM_DEPTH", "3"))
    # LOGMM_DIAG=noload: hoist all loads out of the repeat loop (needs
    # LOGMM_XIN=16) — isolates the compute+store pipeline for HW timing
    DIAG = os.environ.get("LOGMM_DIAG", "")

    def body(_i=None):
        next_load = 0

        def ensure_x(up_to):
            nonlocal next_load
            if DIAG == "noload":
                return
            while next_load <= min(up_to, S_TILES - 1):
                load_x(next_load)
                next_load += XCH

        ensure_x(1)
        if DIAG != "noload":
            load_matrix()
        for st in range(DEPTH):
            ensure_x(st + 2)
            emit_transposes(st)
        for st in range(S_TILES):
            t_st = st + DEPTH if st + DEPTH < S_TILES else None
            if t_st is not None:
                ensure_x(t_st + 2)
            emit_mms(st, t_st=t_st)

    if DIAG == "noload":
        for st in range(0, S_TILES, XCH):
            load_x(st)
        load_matrix()

        # x_tiles entries are popped by the transposes each iteration; keep a
        # pristine copy to re-seed per iteration
        x_tiles_full = dict(x_tiles)

        def body_noload(_i=None):
            x_tiles.clear()
            x_tiles.update(x_tiles_full)
            body(_i)

        if REPEAT > 1:
            with tc.For_i(0, REPEAT, 1) as _i:
                body_noload(_i)
        else:
            body_noload()
    elif REPEAT > 1:
        with tc.For_i(0, REPEAT, 1) as _i:
            body(_i)
    else:
        body()


def _build_nc(mm_dt: str):
    nc = bacc.Bacc("TRN2", target_bir_lowering=False, debug=False)
    x = nc.dram_tensor("x", [S, K], mybir.dt.float32, kind="ExternalInput").ap()
    mat = nc.dram_tensor("matrix", [K, M], mybir.dt.float32, kind="ExternalInput").ap()
    out = nc.dram_tensor("out", [S, M], mybir.dt.float32, kind="ExternalOutput").ap()
    with tile.TileContext(nc) as tc:
        with ExitStack() as ctx:
            if mm_dt == "fp8dr":
                _emit_fp8(ctx, tc, out, x, mat)
            else:
                _emit(ctx, tc, out, x, mat, mm_dt)
    nc.compile()
    return nc


_nc_cache: dict = {}


def _get_nc(mm_dt: str):
    if mm_dt not in _nc_cache:
        _nc_cache[mm_dt] = _build_nc(mm_dt)
    return _nc_cache[mm_dt]


def kernel(x: np.ndarray, matrix: np.ndarray, _trace: bool = False):
    assert x.shape == (B, S, K) and matrix.shape == (K, M)
    nc = _get_nc(MM_DT)
    x = np.ascontiguousarray(x, dtype=np.float32)
    matrix = np.ascontiguousarray(matrix, dtype=np.float32)
    in_maps = [{"x": x[b], "matrix": matrix} for b in range(N_CORES)]
    res = run_bass_kernel_spmd(nc, in_maps, core_ids=list(range(N_CORES)), trace=_trace)
    out = np.stack([r["out"] for r in res.results], axis=0)
    if _trace:
        kernel.last_results = res  # stash for profiling inspection
    return out



# revision 19
# speedup vs baseline: 1.5063x; 1.3829x over previous
"""Trainium2 Bass kernel for nn_LogMM: out = log(max(x @ matrix, tiny)).

Reference math: y = einsum('bsk,km->bsm', x, matrix); big = (y>0); small = 1-big;
out = log(max(y,eps))*big + log(max(y,eps))*small == log(max(y, eps)).
(y_big == y_small numerically, and big+small == 1 elementwise.)

Sharding: data-parallel over batch B=8, one batch slice per NeuronCore;
matrix replicated. Zero communication.

Per-core kernel: x_b [2048, 1024] @ matrix [1024, 1024] -> log -> out_b.
The contraction dim k must live on SBUF partitions for both matmul operands;
matrix is already [k, m], x tiles are transposed on-chip via PE transpose.
"""

import os
from contextlib import ExitStack

import numpy as np

import concourse.bass as bass
import concourse.bacc as bacc
import concourse.mybir as mybir
import concourse.tile as tile
from concourse.bass_utils import run_bass_kernel_spmd
from concourse.masks import make_identity

B, S, K, M = 8, 2048, 1024, 1024
P = 128
N_CORES = 8

# matmul input dtype: "fp8dr" (fp8e4m3 DoubleRow, 0.5 cyc/row), "f32" (exact,
# 4 cyc/row), "f32r" (fp32 bits, 1 cyc/row at N>=256), "bf16" (cast, 1 cyc/row)
MM_DT = os.environ.get("LOGMM_DT", "fp8dr")
N_TILE = 512
# timing aid: repeat the whole per-core computation R times inside the NEFF
REPEAT = int(os.environ.get("LOGMM_REPEAT", "1"))


def _emit(ctx: ExitStack, tc: "tile.TileContext", out_ap, x_ap, mat_ap, mm_dt: str):
    nc = tc.nc
    S_TILES = S // P  # 16
    KO = K // P  # 8
    MO = M // N_TILE

    # dtype of the SBUF tiles fed to the accumulation matmuls
    if mm_dt == "bf16":
        mm_sb_dt = mybir.dt.bfloat16
    elif mm_dt == "f32r":
        mm_sb_dt = mybir.dt.float32r
    else:
        mm_sb_dt = mybir.dt.float32

    # x tiles are loaded [s, k] and transposed on PE; the transpose runs in
    # the load dtype (fp32 for f32/f32r, bf16 for bf16 via casting DMA).
    # LOGMM_TDT=f32r additionally runs the transposes themselves in f32r
    # (1.5 vs 2.0 cyc/row) by loading x as f32r via casting DMA.
    if mm_dt == "bf16":
        ld_dt = mybir.dt.bfloat16
    elif mm_dt == "f32r" and os.environ.get("LOGMM_TDT", "f32r") == "f32r":
        ld_dt = mybir.dt.float32r
    else:
        ld_dt = mybir.dt.float32

    const_pool = ctx.enter_context(tc.tile_pool(name="const", bufs=1))
    xin_pool = ctx.enter_context(tc.tile_pool(name="xin", bufs=int(os.environ.get("LOGMM_XIN","6"))))
    xt_pool = ctx.enter_context(tc.tile_pool(name="xt", bufs=int(os.environ.get("LOGMM_XT","5"))))
    ob_pool = ctx.enter_context(tc.tile_pool(name="ob", bufs=4))
    pst_pool = ctx.enter_context(tc.tile_pool(name="pst", bufs=int(os.environ.get("LOGMM_PST","4")), space="PSUM"))
    psm_pool = ctx.enter_context(tc.tile_pool(name="psm", bufs=int(os.environ.get("LOGMM_PSM","4")), space="PSUM"))

    if ld_dt == mybir.dt.float32r:
        # affine_select can't produce f32r; build fp32 identity and DVE-round
        ident_f32 = const_pool.tile([P, P], mybir.dt.float32)
        make_identity(nc, ident_f32)
        ident = const_pool.tile([P, P], ld_dt)
        nc.vector.tensor_copy(ident[:], ident_f32[:])
    else:
        ident = const_pool.tile([P, P], ld_dt)
        make_identity(nc, ident)

    mat_sb = const_pool.tile([P, KO, M], mm_sb_dt)
    mat_src = mat_ap.rearrange("(ko p) m -> p ko m", p=P)
    x_tiles: dict = {}

    def load_x(st, chunks=1):
        x_nat = xin_pool.tile([P, K], ld_dt)  # s on partitions, k free
        dma = (nc.scalar if os.environ.get("LOGMM_XQ","sp")=="act" else nc.sync) if ld_dt != mybir.dt.bfloat16 else nc.gpsimd
        if ld_dt == mybir.dt.float32r:
            x_ap_ld = x_ap.bitcast(mybir.dt.float32r)
        else:
            x_ap_ld = x_ap
        cw = K // chunks
        for c in range(chunks):
            dma.dma_start(
                x_nat[:, c * cw : (c + 1) * cw],
                x_ap_ld[st * P : (st + 1) * P, c * cw : (c + 1) * cw],
            )
        x_tiles[st] = x_nat

    def load_matrix():
        # matrix -> SBUF [P(k_inner), KO(k_outer), M]; chunked per ko so the
        # first matmuls aren't gated on the full 4MB transfer.
        if mm_sb_dt == mybir.dt.float32r and os.environ.get("LOGMM_MBC", "1") == "1":
            # bitcast the DRAM source to f32r and DMA straight into mat_sb:
            # drops the fp32 staging buffer and 16 DVE rounding copies from
            # each matmul's wait chain (PE truncates f32r on ingest anyway)
            mat_src_r = mat_src.bitcast(mybir.dt.float32r)
            for ko in range(KO):
                for h in range(2):
                    h_sl = slice(h * (M // 2), (h + 1) * (M // 2))
                    nc.sync.dma_start(mat_sb[:, ko, h_sl], mat_src_r[:, ko, h_sl])
        elif mm_sb_dt == mybir.dt.float32r:
            mat_stage = const_pool.tile([P, KO, M], mybir.dt.float32)
            for ko in range(KO):
                for h in range(2):
                    h_sl = slice(h * (M // 2), (h + 1) * (M // 2))
                    nc.sync.dma_start(mat_stage[:, ko, h_sl], mat_src[:, ko, h_sl])
                    # rounds fp32 -> fp32r as required by the BIR verifier
                    nc.vector.tensor_copy(mat_sb[:, ko, h_sl], mat_stage[:, ko, h_sl])
        else:
            dma = nc.sync if mm_sb_dt == mybir.dt.float32 else nc.gpsimd
            for ko in range(KO):
                dma.dma_start(mat_sb[:, ko, :], mat_src[:, ko, :])

    xT_tiles: dict = {}
    TB = 512 // P  # transposes per PSUM bank

    def transpose_batch(st, kb):
        # transpose 4 128x128 blocks of x tile st into one PSUM bank, then one
        # [128,512] PSUM->SBUF copy (which also rounds to the matmul dtype).
        x_nat = x_tiles[st]
        if st not in xT_tiles:
            xT_tiles[st] = xt_pool.tile([P, KO, P], mm_sb_dt, name="xT", tag="xT")
        xT = xT_tiles[st]
        ps = pst_pool.tile([P, TB, P], ld_dt)
        for kt in range(TB):
            ko = kb * TB + kt
            nc.tensor.transpose(
                ps[:, kt, :], x_nat[:, ko * P : (ko + 1) * P], ident[:]
            )
        nc.vector.tensor_copy(xT[:, kb * TB : (kb + 1) * TB, :], ps[:])
        if kb == KO // TB - 1:
            x_tiles.pop(st)

    def emit_transposes(st):
        for kb in range(KO // TB):
            transpose_batch(st, kb)

    def emit_mms(st, mo_inner, t_st=None, last=False):
        s_sl = slice(st * P, (st + 1) * P)
        xT = xT_tiles.pop(st)
        # transpose batches for tile t_st, interleaved into this MM stream so
        # PE can fill waits (matrix pacing early on, psum/DVE waits later)
        fillers = (
            [(t_st, kb) for kb in range(KO // TB)] if t_st is not None else []
        )

        def filler(ko):
            if fillers and ko % 2 == 1:
                transpose_batch(*fillers.pop(0))

        def fin(mo, pm):
            m_sl = slice(mo * N_TILE, (mo + 1) * N_TILE)
            ob = ob_pool.tile([P, N_TILE], mybir.dt.float32)
            nc.scalar.activation(ob[:], pm[:], mybir.ActivationFunctionType.Ln)
            nc.sync.dma_start(out_ap[s_sl, m_sl], ob[:])

        if mo_inner:
            # each matmul gates on a single matrix ko-chunk (matters for the
            # first s-tiles while the matrix is still streaming in)
            pms = [
                psm_pool.tile([P, N_TILE], mybir.dt.float32, name=f"pm{mo}", tag="pm")
                for mo in range(MO)
            ]
            for ko in range(KO):
                for mo in range(MO):
                    nc.tensor.matmul(
                        pms[mo][:],
                        xT[:, ko, :],
                        mat_sb[:, ko, mo * N_TILE : (mo + 1) * N_TILE],
                        start=(ko == 0),
                        stop=(ko == KO - 1),
                    )
                filler(ko)
            for mo in range(MO):
                fin(mo, pms[mo])
        else:
            # mo-outer: each psum finishes asap so log+store drain earlier
            for mo in range(MO):
                pm = psm_pool.tile([P, N_TILE], mybir.dt.float32, tag="pm")
                for ko in range(KO):
                    nc.tensor.matmul(
                        pm[:],
                        xT[:, ko, :],
                        mat_sb[:, ko, mo * N_TILE : (mo + 1) * N_TILE],
                        start=(ko == 0),
                        stop=(ko == KO - 1),
                    )
                    filler(mo * KO + ko)
                fin(mo, pm)

    DEPTH = int(os.environ.get("LOGMM_DEPTH", "3"))

    def body(_i=None):  # noqa: C901
        next_load = 0

        def ensure_x(up_to):
            nonlocal next_load
            while next_load <= min(up_to, S_TILES - 1):
                # first tiles in small chunks so the first transposes start asap
                load_x(next_load)
                next_load += 1

        # first x tiles before the matrix so PE transposes start immediately
        ensure_x(1)
        load_matrix()
        # HAM pre-warm: the PE would otherwise idle ~4us waiting for x tile 0
        # and then run its first ~3.4us of real work at the cold 1.2 GHz clock.
        # Dummy transposes of the (already resident) identity keep the PE busy
        # through the DMA wait so real work starts at 2.4 GHz.
        n_warm = int(os.environ.get("LOGMM_WARM", "0"))
        if n_warm:
            ps_w = pst_pool.tile([P, TB, P], ld_dt, name="ps_warm", tag="ps")
            for w in range(n_warm):
                nc.tensor.transpose(ps_w[:, w % TB, :], ident[:], ident[:])
            # consume the last dummy so DCE keeps the chain: store one row
            # into out[0:P, 0:P], which s-tile 0's real store later overwrites
            warm_sb = ob_pool.tile([P, P], ld_dt, name="warm_sb")
            nc.vector.tensor_copy(warm_sb[:], ps_w[:, 0, :])
            nc.sync.dma_start(
                out_ap[0:P, 0:P], warm_sb[:].bitcast(mybir.dt.float32)
            )
        for st in range(DEPTH):
            ensure_x(st + 2)
            emit_transposes(st)
        for st in range(S_TILES):
            t_st = st + DEPTH if st + DEPTH < S_TILES else None
            if t_st is not None:
                ensure_x(t_st + 2)
            emit_mms(st, mo_inner=st < int(os.environ.get("LOGMM_MOI","2")), t_st=t_st, last=st >= S_TILES - 2)

    if REPEAT > 1:
        with tc.For_i(0, REPEAT, 1) as _i:
            body(_i)
    else:
        body()


def _emit_fp8(ctx: ExitStack, tc: "tile.TileContext", out_ap, x_ap, mat_ap):
    """fp8e4m3 DoubleRow pipeline.

    x, matrix are cast fp32->{bf16,fp8} inline by SWDGE (gpsimd) DMAs, which
    also moves all loads off the HWDGE ring (stores keep it). x tiles are
    PE-transposed (1 cyc/row at 16-bit/8-bit), DVE-copied into fp8 xT tiles,
    then each s-tile runs KP=4 DoubleRow matmuls per 512-wide output half:
    contraction 256 per matmul via the [ki, 2, *] interleaved APs on both
    operands. PSUM accumulates fp32; ACT applies Ln; sync HWDGE stores.
    """
    nc = tc.nc
    S_TILES = S // P  # 16
    KO = K // P  # 8
    KP = KO // 2  # DoubleRow k-pair groups
    MO = M // N_TILE

    mm_dt = mybir.dt.float8e4
    # x load dtype:
    #  - f32r (default): plain HWDGE load on the sync ring, PE transpose at
    #    1.5 cyc/row, fp8 conversion folded into the DVE PSUM->SBUF copy.
    #    Keeps the Q7/SWDGE descriptor engine (which shares an SBUF port
    #    with DVE) out of the x path entirely.
    #  - bf16/fp8: SWDGE casting DMA on gpsimd, 1 cyc/row transpose.
    XDT = os.environ.get("LOGMM_XDT", "f32r")
    ld_dt = {
        "fp8": mybir.dt.float8e4,
        "bf16": mybir.dt.bfloat16,
        "f32r": mybir.dt.float32r,
    }[XDT]

    const_pool = ctx.enter_context(tc.tile_pool(name="const", bufs=1))
    # matrix double-buffered across REPEAT iterations: iteration i+1's reload
    # must not WAR-stall on iteration i's last matmuls
    mat_pool = ctx.enter_context(
        tc.tile_pool(name="matp", bufs=int(os.environ.get("LOGMM_MATB", "2")))
    )
    xin_pool = ctx.enter_context(
        tc.tile_pool(name="xin", bufs=int(os.environ.get("LOGMM_XIN", "8")))
    )
    xt_pool = ctx.enter_context(
        tc.tile_pool(name="xt", bufs=int(os.environ.get("LOGMM_XT", "5")))
    )
    ob_pool = ctx.enter_context(
        tc.tile_pool(name="ob", bufs=int(os.environ.get("LOGMM_OB", "4")))
    )
    pst_pool = ctx.enter_context(
        tc.tile_pool(name="pst", bufs=int(os.environ.get("LOGMM_PST", "3")), space="PSUM")
    )
    psm_pool = ctx.enter_context(
        tc.tile_pool(name="psm", bufs=int(os.environ.get("LOGMM_PSM", "5")), space="PSUM")
    )

    if ld_dt == mybir.dt.float32r:
        # affine_select can't produce f32r; build fp32 identity and DVE-round
        ident_f32 = const_pool.tile([P, P], mybir.dt.float32)
        make_identity(nc, ident_f32)
        ident = const_pool.tile([P, P], ld_dt)
        nc.vector.tensor_copy(ident[:], ident_f32[:])
    else:
        ident = const_pool.tile([P, P], ld_dt)
        make_identity(nc, ident)

    mat_src = mat_ap.rearrange("(ko p) m -> p ko m", p=P)
    x_tiles: dict = {}
    mat_sb = None
    stq = os.environ.get("LOGMM_STQ", "scalar" if XDT == "f32r" else "sync")
    store_engines = {
        "alt": (nc.sync, nc.scalar),
        "scalar": (nc.scalar,),
        "sync": (nc.sync,),
    }[stq]

    XCH = int(os.environ.get("LOGMM_XCH", "1"))  # s-tiles per x load DMA
    x_ap_ld = (
        x_ap.bitcast(mybir.dt.float32r) if ld_dt == mybir.dt.float32r else x_ap
    )
    x_src_g = x_ap_ld.rearrange("(st g p) k -> st p g k", p=P, g=XCH)

    def load_x(st0):
        # one DMA covering XCH s-tiles: [128, XCH, K]; slice g recovers
        # s-tile st0+g. f32r goes over the sync HWDGE ring (no cast); the
        # 16-bit/8-bit dtypes need the SWDGE (gpsimd) casting path.
        x_nat = xin_pool.tile([P, XCH, K], ld_dt)
        if ld_dt == mybir.dt.float32r:
            dma = nc.scalar if os.environ.get("LOGMM_XLQ", "sync") == "scalar" else nc.sync
        else:
            dma = nc.gpsimd
        dma.dma_start(x_nat[:], x_src_g[st0 // XCH])
        for g in range(XCH):
            x_tiles[st0 + g] = x_nat[:, g, :]

    MATLD = os.environ.get("LOGMM_MATLD", "swdge")
    mat_stage_pool = (
        ctx.enter_context(tc.tile_pool(name="mats", bufs=2))
        if MATLD == "hwdge"
        else None
    )

    def load_matrix():
        # fp32 -> fp8; chunked so the first matmuls only gate on the ko-pairs
        # they consume. swdge: casting DMA on gpsimd. hwdge: plain fp32 load
        # on the sync ring + DVE conversion (keeps Q7/SWDGE fully idle).
        nonlocal mat_sb
        mat_sb = mat_pool.tile([P, KO, M], mm_dt, name="mat", tag="mat")
        mch = int(os.environ.get("LOGMM_MATCHUNK", "8"))
        per = KO // mch
        for c in range(mch):
            c_sl = slice(c * per, (c + 1) * per)
            if MATLD == "hwdge":
                stage = mat_stage_pool.tile([P, per, M], mybir.dt.float32)
                nc.sync.dma_start(stage[:], mat_src[:, c_sl, :])
                nc.vector.tensor_copy(mat_sb[:, c_sl, :], stage[:])
            else:
                nc.gpsimd.dma_start(mat_sb[:, c_sl, :], mat_src[:, c_sl, :])

    xT_tiles: dict = {}
    TB = 512 // P  # transposes per PSUM bank

    def transpose_batch(st, kb):
        x_nat = x_tiles[st]
        if st not in xT_tiles:
            xT_tiles[st] = xt_pool.tile([P, KO, P], mm_dt, name="xT", tag="xT")
        xT = xT_tiles[st]
        ps = pst_pool.tile([P, TB, P], ld_dt)
        for kt in range(TB):
            ko = kb * TB + kt
            nc.tensor.transpose(
                ps[:, kt, :], x_nat[:, ko * P : (ko + 1) * P], ident[:]
            )
        nc.vector.tensor_copy(xT[:, kb * TB : (kb + 1) * TB, :], ps[:])
        if kb == KO // TB - 1:
            x_tiles.pop(st)

    def emit_transposes(st):
        for kb in range(KO // TB):
            transpose_batch(st, kb)

    def emit_mms(st, t_st=None):
        s_sl = slice(st * P, (st + 1) * P)
        xT = xT_tiles.pop(st)
        fillers = (
            [(t_st, kb) for kb in range(KO // TB)] if t_st is not None else []
        )

        def filler():
            if fillers:
                transpose_batch(*fillers.pop(0))

        pms = [
            psm_pool.tile([P, N_TILE], mybir.dt.float32, name=f"pm{mo}", tag="pm")
            for mo in range(MO)
        ]
        for j in range(KP):
            lhsT = xT[:, 2 * j : 2 * j + 2, :]
            for mo in range(MO):
                nc.tensor.matmul(
                    pms[mo][:],
                    lhsT,
                    mat_sb[:, 2 * j : 2 * j + 2, mo * N_TILE : (mo + 1) * N_TILE],
                    start=(j == 0),
                    stop=(j == KP - 1),
                    perf_mode=mybir.MatmulPerfMode.DoubleRow,
                )
            if j % 2 == 1:
                filler()
        if os.environ.get("LOGMM_STMERGE", "0") == "1":
            # both Ln halves into one [P, M] tile, single 512KB store per
            # s-tile — halves the HWDGE store issue count
            ob = ob_pool.tile([P, M], mybir.dt.float32)
            for mo in range(MO):
                m_sl = slice(mo * N_TILE, (mo + 1) * N_TILE)
                nc.scalar.activation(
                    ob[:, m_sl], pms[mo][:], mybir.ActivationFunctionType.Ln
                )
            store_engines[st % len(store_engines)].dma_start(out_ap[s_sl, :], ob[:])
        else:
            for mo in range(MO):
                m_sl = slice(mo * N_TILE, (mo + 1) * N_TILE)
                ob = ob_pool.tile([P, N_TILE], mybir.dt.float32)
                nc.scalar.activation(
                    ob[:], pms[mo][:], mybir.ActivationFunctionType.Ln
                )
                store_engines[(st * MO + mo) % len(store_engines)].dma_start(
                    out_ap[s_sl, m_sl], ob[:]
                )

    DEPTH = int(os.environ.get("LOGMM_DEPTH", "3"))
    # LOGMM_DIAG=noload: hoist all loads out of the repeat loop (needs
    # LOGMM_XIN=16) — isolates the compute+store pipeline for HW timing
    DIAG = os.environ.get("LOGMM_DIAG", "")

    def body(_i=None):
        next_load = 0

        def ensure_x(up_to):
            nonlocal next_load
            if DIAG == "noload":
                return
            while next_load <= min(up_to, S_TILES - 1):
                load_x(next_load)
                next_load += XCH

        ensure_x(1)
        if DIAG != "noload":
            load_matrix()
        for st in range(DEPTH):
            ensure_x(st + 2)
            emit_transposes(st)
        for st in range(S_TILES):
            t_st = st + DEPTH if st + DEPTH < S_TILES else None
            if t_st is not None:
                ensure_x(t_st + 2)
            emit_mms(st, t_st=t_st)

    if DIAG == "noload":
        for st in range(0, S_TILES, XCH):
            load_x(st)
        load_matrix()

        # x_tiles entries are popped by the transposes each iteration; keep a
        # pristine copy to re-seed per iteration
        x_tiles_full = dict(x_tiles)

        def body_noload(_i=None):
            x_tiles.clear()
            x_tiles.update(x_tiles_full)
            body(_i)

        if REPEAT > 1:
            with tc.For_i(0, REPEAT, 1) as _i:
                body_noload(_i)
        else:
            body_noload()
    elif REPEAT > 1:
        with tc.For_i(0, REPEAT, 1) as _i:
            body(_i)
    else:
        body()


def _build_nc(mm_dt: str):
    nc = bacc.Bacc("TRN2", target_bir_lowering=False, debug=False)
    x = nc.dram_tensor("x", [S, K], mybir.dt.float32, kind="ExternalInput").ap()
    mat = nc.dram_tensor("matrix", [K, M], mybir.dt.float32, kind="ExternalInput").ap()
    out = nc.dram_tensor("out", [S, M], mybir.dt.float32, kind="ExternalOutput").ap()
    with tile.TileContext(nc) as tc:
        with ExitStack() as ctx:
            if mm_dt == "fp8dr":
                _emit_fp8(ctx, tc, out, x, mat)
            else:
                _emit(ctx, tc, out, x, mat, mm_dt)
    nc.compile()
    return nc


_nc_cache: dict = {}


def _get_nc(mm_dt: str):
    if mm_dt not in _nc_cache:
        _nc_cache[mm_dt] = _build_nc(mm_dt)
    return _nc_cache[mm_dt]


def kernel(x: np.ndarray, matrix: np.ndarray, _trace: bool = False):
    assert x.shape == (B, S, K) and matrix.shape == (K, M)
    nc = _get_nc(MM_DT)
    x = np.ascontiguousarray(x, dtype=np.float32)
    matrix = np.ascontiguousarray(matrix, dtype=np.float32)
    in_maps = [{"x": x[b], "matrix": matrix} for b in range(N_CORES)]
    res = run_bass_kernel_spmd(nc, in_maps, core_ids=list(range(N_CORES)), trace=_trace)
    out = np.stack([r["out"] for r in res.results], axis=0)
    if _trace:
        kernel.last_results = res  # stash for profiling inspection
    return out

